# revision 11
# baseline (speedup 1.0000x reference)
"""Trainium2 Bass kernel for nn_Attn_33054068310077 (Bahdanau-style attention scores).

Reference math:
    energy = concat([broadcast(hidden), enc], -1) @ W.T + b   # [B,S,H]
    scores = energy @ v                                       # [B,S]
    out    = softmax(scores, axis=-1)[:, None, :]             # [B,1,S]

Weight folding (exact up to fp reassociation):
    scores[b,s] = enc[b,s,:] @ u  +  (hidden[b,0,:] @ (v @ W[:, :H]) + b @ v)
    with u = v @ W[:, H:].
The second term does not depend on s, so softmax cancels it exactly:
    out = softmax(enc @ u, axis=-1),   u = v @ W[:, H:2H].

Device kernel (SPMD, 8 NeuronCores, data-parallel over batch, 2 batches/core):
    - stream enc in [128, 1024] tiles (512 KB contiguous DMA each)
    - fused multiply + row-sum per tile on VectorE (tensor_tensor_reduce)
    - softmax with true max: free-axis max on VectorE, cross-partition max on
      GpSimd, exp + row-sum on ScalarE, cross-partition sum via PE ones-matmul,
      reciprocal on VectorE, PE transpose for a contiguous output DMA.
"""

import numpy as np

B, S, H = 16, 2048, 1024
NCORES = 8
BPC = B // NCORES          # batches per core
P = 128                    # SBUF partitions
NCHUNKS = S // P           # 16 s-chunks per batch
TILES = BPC * NCHUNKS      # 32 tiles per core

_CACHE = {}
LAST_RESULT = None         # BassKernelResults of the most recent run (for test.py)


def _build_nc():
    import concourse.bacc as bacc
    import concourse.bass as bass
    import concourse.tile as tile
    from concourse import mybir


    f32 = mybir.dt.float32
    nc = bacc.Bacc(None, target_bir_lowering=False)

    enc = nc.dram_tensor("enc", [BPC, S, H], f32, kind="ExternalInput")
    u = nc.dram_tensor("u", [H], f32, kind="ExternalInput")
    ident = nc.dram_tensor("ident", [P, P], f32, kind="ExternalInput")
    out = nc.dram_tensor("out", [BPC, NCHUNKS, P], f32, kind="ExternalOutput")

    with tile.TileContext(nc) as tc:
        with (
            tc.tile_pool(name="consts", bufs=1) as consts,
            tc.tile_pool(name="encp", bufs=8) as encp,
            tc.tile_pool(name="scorep", bufs=1) as scorep,
            tc.tile_pool(name="small", bufs=8) as small,
            tc.tile_pool(name="expp", bufs=2) as expp,
            tc.tile_pool(name="outp", bufs=2) as outp,
            tc.tile_pool(name="psum1", bufs=1, space="PSUM") as psum1,
            tc.tile_pool(name="psum2", bufs=2, space="PSUM") as psum2,
        ):
            # constants go through the gpsimd DMA queue so the sync queue's first
            # issues are enc tiles
            idt = consts.tile([P, P], f32)
            nc.gpsimd.dma_start(out=idt[:], in_=ident[:])
            ones_col = consts.tile([P, 1], f32)
            nc.vector.memset(ones_col[:], 1.0)
            ones_row = consts.tile([1, P], f32)
            nc.vector.memset(ones_row[:], 1.0)
            # u: 4 KB DMA to one partition, then PE ones-matmul broadcast to all 128
            # (avoids a 512 KB broadcast DMA competing with the enc stream)
            u_sb = consts.tile([1, H], f32)
            u_ap = u[:]
            nc.gpsimd.dma_start(
                out=u_sb[:],
                in_=bass.AP(tensor=u_ap.tensor, offset=u_ap.offset, ap=[[0, 1], *u_ap.ap]),
            )
            ub = consts.tile([P, H], f32)
            for ci in range(H // 512):
                pu = psum2.tile([P, 512], f32, tag="pu")
                nc.tensor.matmul(
                    pu[:], lhsT=ones_row[:], rhs=u_sb[0:1, ci * 512 : (ci + 1) * 512],
                    start=True, stop=True,
                )
                nc.scalar.copy(out=ub[:, ci * 512 : (ci + 1) * 512], in_=pu[:])
            # Prewarm the exp table set so ACT_TABLE_LOAD overlaps the DMA phase.
            warm = consts.tile([1, 1], f32)
            nc.vector.memset(warm[:], 0.0)
            nc.scalar.activation(
                out=warm[:], in_=warm[:], func=mybir.ActivationFunctionType.Exp
            )

            scores = scorep.tile([P, TILES], f32)

            def emit_chunk(b, c):
                t = b * NCHUNKS + c
                et = encp.tile([P, H], f32, tag="et")
                nc.sync.dma_start(out=et[:], in_=enc[b, c * P : (c + 1) * P, :])
                # scores[:, t] = sum_h et[:, h] * u[h]   (product kept in-place;
                # one DVE pass: out = (in0 * 1.0) * in1, accum_out = row-sum)
                nc.vector.scalar_tensor_tensor(
                    out=et[:],
                    in0=et[:],
                    scalar=1.0,
                    in1=ub[:],
                    op0=mybir.AluOpType.mult,
                    op1=mybir.AluOpType.mult,
                    accum_out=scores[:, t : t + 1],
                )

            def softmax_steps(b):
                """Generator of softmax pipeline steps for batch b; emitted
                interleaved with the next batch's chunk jobs so the short DVE
                ops (with cross-engine waits between them) don't stall the
                in-order DVE stream of scalar_tensor_tensors."""
                sc = scores[:, b * NCHUNKS : (b + 1) * NCHUNKS]
                st = {}
                # per-partition max, then cross-partition max via PE transpose
                mx = small.tile([P, 1], f32, tag="mx")
                nc.vector.tensor_reduce(
                    out=mx[:], in_=sc, axis=mybir.AxisListType.X, op=mybir.AluOpType.max
                )
                pmx = psum1.tile([1, P], f32, tag="pmx")
                nc.tensor.transpose(pmx[:], mx[:], idt[:])
                yield
                negmg = small.tile([1, 1], f32, tag="negmg")
                nc.vector.tensor_reduce(
                    out=negmg[:], in_=pmx[:], axis=mybir.AxisListType.X,
                    op=mybir.AluOpType.max, negate=True,
                )
                # broadcast -max to all partitions (ones-matmul), land in SBUF
                pneg = psum1.tile([P, 1], f32, tag="pneg")
                nc.tensor.matmul(pneg[:], lhsT=ones_row[:], rhs=negmg[:], start=True, stop=True)
                negm = small.tile([P, 1], f32, tag="negm")
                nc.scalar.copy(out=negm[:], in_=pneg[:])
                yield
                expb = expp.tile([P, NCHUNKS], f32, tag="expb")
                sums = small.tile([P, 1], f32, tag="sums")
                nc.scalar.activation(
                    out=expb[:],
                    in_=sc,
                    func=mybir.ActivationFunctionType.Exp,
                    bias=negm[:],
                    scale=1.0,
                    accum_out=sums[:],
                )
                # total = sum over partitions of sums  (PE contracts partitions)
                ptot = psum1.tile([1, 1], f32, tag="ptot")
                nc.tensor.matmul(ptot[:], lhsT=sums[:], rhs=ones_col[:], start=True, stop=True)
                yield
                rtot = small.tile([1, 1], f32, tag="rtot")
                nc.vector.reciprocal(rtot[:], ptot[:])
                # broadcast 1/total back to all partitions; DVE reads the
                # per-partition scalar straight from PSUM
                pbc = psum1.tile([P, 1], f32, tag="pbc")
                nc.tensor.matmul(pbc[:], lhsT=ones_row[:], rhs=rtot[:], start=True, stop=True)
                yield
                nc.vector.tensor_scalar_mul(expb[:], expb[:], pbc[:])
                # transpose [128, 16] -> [16, 128] so the output DMA is contiguous
                pT = psum2.tile([NCHUNKS, P], f32, tag="pT")
                nc.tensor.transpose(pT[:], expb[:], idt[:])
                yield
                outT = outp.tile([NCHUNKS, P], f32, tag="outT")
                nc.scalar.copy(out=outT[:], in_=pT[:])
                nc.sync.dma_start(out=out[b], in_=outT[:])

            pending = []
            for b in range(BPC):
                for c in range(NCHUNKS):
                    emit_chunk(b, c)
                    if pending and c % 2 == 1:
                        for g in list(pending):
                            if next(g, "done") == "done":
                                pending.remove(g)
                            break
                pending.append(softmax_steps(b))
            # drain remaining softmax steps (tail of the last batch)
            for g in pending:
                for _ in g:
                    pass

    nc.compile()
    return nc


def _get_nc():
    if "nc" not in _CACHE:
        _CACHE["nc"] = _build_nc()
    return _CACHE["nc"]


def kernel(hidden, encoder_outputs, attn_w, attn_b, v, _trace=False, _trace_kwargs=None):
    global LAST_RESULT
    from concourse.bass_utils import run_bass_kernel_spmd

    encoder_outputs = np.ascontiguousarray(np.asarray(encoder_outputs, dtype=np.float32))
    attn_w = np.asarray(attn_w, dtype=np.float32)
    v = np.asarray(v, dtype=np.float32)
    assert encoder_outputs.shape == (B, S, H)

    # Host-side weight fold: u = v @ W[:, H:]  (the hidden/bias terms cancel in softmax)
    u = np.ascontiguousarray(v[0] @ attn_w[:, H:]).astype(np.float32)
    ident = np.eye(P, dtype=np.float32)

    in_maps = [
        {
            "enc": np.ascontiguousarray(encoder_outputs[i * BPC : (i + 1) * BPC]),
            "u": u,
            "ident": ident,
        }
        for i in range(NCORES)
    ]

    nc = _get_nc()
    kwargs = {}
    if _trace:
        kwargs["trace"] = True
        if _trace_kwargs:
            kwargs.update(_trace_kwargs)
    LAST_RESULT = run_bass_kernel_spmd(nc, in_maps, core_ids=list(range(NCORES)), **kwargs)

    outs = [LAST_RESULT.results[i]["out"].reshape(BPC, S) for i in range(NCORES)]
    full = np.concatenate(outs, axis=0)          # [B, S]
    return full[:, None, :].astype(np.float32)   # [B, 1, S]


# revision 14
# speedup vs baseline: 1.0919x; 1.0919x over previous
"""Trainium2 Bass kernel for nn_Attn_33054068310077 (Bahdanau-style attention scores).

Reference math:
    energy = concat([broadcast(hidden), enc], -1) @ W.T + b   # [B,S,H]
    scores = energy @ v                                       # [B,S]
    out    = softmax(scores, axis=-1)[:, None, :]             # [B,1,S]

Weight folding (exact up to fp reassociation):
    scores[b,s] = enc[b,s,:] @ u  +  (hidden[b,0,:] @ (v @ W[:, :H]) + b @ v)
    with u = v @ W[:, H:].
The second term does not depend on s, so softmax cancels it exactly:
    out = softmax(enc @ u, axis=-1),   u = v @ W[:, H:2H].

Device kernel (SPMD, 8 NeuronCores, data-parallel over batch, 2 batches/core):
    - stream enc in [128, 1024] tiles (512 KB contiguous DMA each)
    - fused multiply + row-sum per tile on VectorE (tensor_tensor_reduce)
    - softmax with true max: free-axis max on VectorE, cross-partition max on
      GpSimd, exp + row-sum on ScalarE, cross-partition sum via PE ones-matmul,
      reciprocal on VectorE, PE transpose for a contiguous output DMA.
"""

import numpy as np

B, S, H = 16, 2048, 1024
NCORES = 8
BPC = B // NCORES          # batches per core
P = 128                    # SBUF partitions
NCHUNKS = S // P           # 16 s-chunks per batch
TILES = BPC * NCHUNKS      # 32 tiles per core

_CACHE = {}
LAST_RESULT = None         # BassKernelResults of the most recent run (for test.py)


def _build_nc():
    import concourse.bacc as bacc
    import concourse.bass as bass
    import concourse.tile as tile
    from concourse import mybir


    f32 = mybir.dt.float32
    nc = bacc.Bacc(None, target_bir_lowering=False)

    enc = nc.dram_tensor("enc", [BPC, S, H], f32, kind="ExternalInput")
    u = nc.dram_tensor("u", [H], f32, kind="ExternalInput")
    ident = nc.dram_tensor("ident", [P, P], f32, kind="ExternalInput")
    out = nc.dram_tensor("out", [BPC, NCHUNKS, P], f32, kind="ExternalOutput")

    with tile.TileContext(nc) as tc:
        with (
            tc.tile_pool(name="consts", bufs=1) as consts,
            tc.tile_pool(name="encp", bufs=12) as encp,
            tc.tile_pool(name="scorep", bufs=1) as scorep,
            tc.tile_pool(name="small", bufs=8) as small,
            tc.tile_pool(name="expp", bufs=2) as expp,
            tc.tile_pool(name="outp", bufs=2) as outp,
            tc.tile_pool(name="psum1", bufs=1, space="PSUM") as psum1,
            tc.tile_pool(name="psum2", bufs=2, space="PSUM") as psum2,
        ):
            # constants go through the gpsimd DMA queue so the sync queue's first
            # issues are enc tiles
            idt = consts.tile([P, P], f32)
            nc.gpsimd.dma_start(out=idt[:], in_=ident[:])
            ones_col = consts.tile([P, 1], f32)
            nc.vector.memset(ones_col[:], 1.0)
            ones_row = consts.tile([1, P], f32)
            nc.vector.memset(ones_row[:], 1.0)
            # u: 4 KB DMA to one partition, then PE ones-matmul broadcast to all 128
            # (avoids a 512 KB broadcast DMA competing with the enc stream)
            u_sb = consts.tile([1, H], f32)
            u_ap = u[:]
            nc.gpsimd.dma_start(
                out=u_sb[:],
                in_=bass.AP(tensor=u_ap.tensor, offset=u_ap.offset, ap=[[0, 1], *u_ap.ap]),
            )
            ub = consts.tile([P, H], f32)
            for ci in range(H // 512):
                pu = psum2.tile([P, 512], f32, tag="pu")
                nc.tensor.matmul(
                    pu[:], lhsT=ones_row[:], rhs=u_sb[0:1, ci * 512 : (ci + 1) * 512],
                    start=True, stop=True,
                )
                nc.scalar.copy(out=ub[:, ci * 512 : (ci + 1) * 512], in_=pu[:])
            # Prewarm the exp table set so ACT_TABLE_LOAD overlaps the DMA phase.
            warm = consts.tile([1, 1], f32)
            nc.vector.memset(warm[:], 0.0)
            nc.scalar.activation(
                out=warm[:], in_=warm[:], func=mybir.ActivationFunctionType.Exp
            )

            scores = scorep.tile([P, TILES], f32)

            def emit_chunk(b, c):
                t = b * NCHUNKS + c
                et = encp.tile([P, H], f32, tag="et")
                nc.sync.dma_start(out=et[:], in_=enc[b, c * P : (c + 1) * P, :])
                # scores[:, t] = sum_h et[:, h] * u[h]   (product kept in-place;
                # one DVE pass: out = (in0 * 1.0) * in1, accum_out = row-sum)
                nc.vector.scalar_tensor_tensor(
                    out=et[:],
                    in0=et[:],
                    scalar=1.0,
                    in1=ub[:],
                    op0=mybir.AluOpType.mult,
                    op1=mybir.AluOpType.mult,
                    accum_out=scores[:, t : t + 1],
                )

            SHIFT_CHUNKS = 12  # shift C_b = max over the first 12 chunks; any
            # consistent C within ~87 of the true max is exact for softmax
            # (exp(s-C) stays finite), so the cross-partition max chain can run
            # while the remaining chunks still stream in.

            negm_tiles = {}

            def shift_steps(b):
                """Compute -C_b broadcast to [P,1] (SBUF) from the first
                SHIFT_CHUNKS chunks of batch b. Hidden under the DMA stream."""
                sc12 = scores[:, b * NCHUNKS : b * NCHUNKS + SHIFT_CHUNKS]
                mx = small.tile([P, 1], f32, tag="mx")
                nc.vector.tensor_reduce(
                    out=mx[:], in_=sc12, axis=mybir.AxisListType.X, op=mybir.AluOpType.max
                )
                pmx = psum1.tile([1, P], f32, tag="pmx")
                nc.tensor.transpose(pmx[:], mx[:], idt[:])
                yield
                negmg = small.tile([1, 1], f32, tag="negmg")
                nc.vector.tensor_reduce(
                    out=negmg[:], in_=pmx[:], axis=mybir.AxisListType.X,
                    op=mybir.AluOpType.max, negate=True,
                )
                pneg = psum1.tile([P, 1], f32, tag="pneg")
                nc.tensor.matmul(pneg[:], lhsT=ones_row[:], rhs=negmg[:], start=True, stop=True)
                yield
                negm = small.tile([P, 1], f32, tag="negm")
                nc.scalar.copy(out=negm[:], in_=pneg[:])
                negm_tiles[b] = negm

            def softmax_steps(b):
                """Exp/normalize/transpose/store for batch b; the per-batch
                shift -C_b is already materialized in negm_tiles[b]."""
                sc = scores[:, b * NCHUNKS : (b + 1) * NCHUNKS]
                expb = expp.tile([P, NCHUNKS], f32, tag="expb")
                sums = small.tile([P, 1], f32, tag="sums")
                nc.scalar.activation(
                    out=expb[:],
                    in_=sc,
                    func=mybir.ActivationFunctionType.Exp,
                    bias=negm_tiles[b][:],
                    scale=1.0,
                    accum_out=sums[:],
                )
                # total = sum over partitions of sums  (PE contracts partitions)
                ptot = psum1.tile([1, 1], f32, tag="ptot")
                nc.tensor.matmul(ptot[:], lhsT=sums[:], rhs=ones_col[:], start=True, stop=True)
                yield
                rtot = small.tile([1, 1], f32, tag="rtot")
                nc.vector.reciprocal(rtot[:], ptot[:])
                # broadcast 1/total back to all partitions; DVE reads the
                # per-partition scalar straight from PSUM
                pbc = psum1.tile([P, 1], f32, tag="pbc")
                nc.tensor.matmul(pbc[:], lhsT=ones_row[:], rhs=rtot[:], start=True, stop=True)
                yield
                nc.vector.tensor_scalar_mul(expb[:], expb[:], pbc[:])
                # transpose [128, 16] -> [16, 128] so the output DMA is contiguous
                pT = psum2.tile([NCHUNKS, P], f32, tag="pT")
                nc.tensor.transpose(pT[:], expb[:], idt[:])
                yield
                outT = outp.tile([NCHUNKS, P], f32, tag="outT")
                nc.scalar.copy(out=outT[:], in_=pT[:])
                nc.sync.dma_start(out=out[b], in_=outT[:])

            pending = []
            for b in range(BPC):
                for c in range(NCHUNKS):
                    emit_chunk(b, c)
                    if c == SHIFT_CHUNKS - 1:
                        pending.append(shift_steps(b))
                    if pending and c % 2 == 1:
                        for g in list(pending):
                            if next(g, "done") == "done":
                                pending.remove(g)
                            break
                pending.append(softmax_steps(b))
            # drain remaining softmax steps (tail of the last batch)
            for g in pending:
                for _ in g:
                    pass

    nc.compile()
    return nc


def _get_nc():
    if "nc" not in _CACHE:
        _CACHE["nc"] = _build_nc()
    return _CACHE["nc"]


def kernel(hidden, encoder_outputs, attn_w, attn_b, v, _trace=False, _trace_kwargs=None):
    global LAST_RESULT
    from concourse.bass_utils import run_bass_kernel_spmd

    encoder_outputs = np.ascontiguousarray(np.asarray(encoder_outputs, dtype=np.float32))
    attn_w = np.asarray(attn_w, dtype=np.float32)
    v = np.asarray(v, dtype=np.float32)
    assert encoder_outputs.shape == (B, S, H)

    # Host-side weight fold: u = v @ W[:, H:]  (the hidden/bias terms cancel in softmax)
    u = np.ascontiguousarray(v[0] @ attn_w[:, H:]).astype(np.float32)
    ident = np.eye(P, dtype=np.float32)

    in_maps = [
        {
            "enc": np.ascontiguousarray(encoder_outputs[i * BPC : (i + 1) * BPC]),
            "u": u,
            "ident": ident,
        }
        for i in range(NCORES)
    ]

    nc = _get_nc()
    kwargs = {}
    if _trace:
        kwargs["trace"] = True
        if _trace_kwargs:
            kwargs.update(_trace_kwargs)
    LAST_RESULT = run_bass_kernel_spmd(nc, in_maps, core_ids=list(range(NCORES)), **kwargs)

    outs = [LAST_RESULT.results[i]["out"].reshape(BPC, S) for i in range(NCORES)]
    full = np.concatenate(outs, axis=0)          # [B, S]
    return full[:, None, :].astype(np.float32)   # [B, 1, S]
